# revision 1
# baseline (speedup 1.0000x reference)
"""Trainium2 Bass kernel for nn_Conv2d (B=32, 256->256, 56x56, 3x3, pad=1) + bias.

Strategy
--------
Data-parallel over batch: 4 images per NeuronCore x 8 cores; weights/bias
replicated; no collectives.

Per core, the conv is computed as shifted matmuls: the input is zero-padded on
the HOST to 58-wide rows (59 rows x 58 cols per image-channel, flattened to
3422), so output position (h, w) <-> flat index h*58+w, and the 3x3 tap
(kh, kw) contribution is a matmul against the padded input shifted by the
constant offset kh*58+kw.  Each output tile [128 couts x 464 positions]
accumulates 2 (cin chunks) x 9 (taps) = 18 matmuls in one PSUM bank
(3248 = 7*464 padded output positions per image; columns w in {56,57} are
junk and stripped on the host).  Matmuls run as float32r (1 cycle/row on the
PE at N>=256; ~1.4e-4 relative error, measured on HW).  Bias is fused into
the PSUM->SBUF eviction via ScalarE activation(Identity, bias=...).
"""

import numpy as np

import concourse.bacc as bacc
import concourse.tile as tile
import concourse.mybir as mybir
from concourse.bass_utils import run_bass_kernel_spmd

F32 = mybir.dt.float32
F32R = mybir.dt.float32r

B, CIN, COUT, H, W, K = 32, 256, 256, 56, 56, 3
NCORES = 8
BPC = B // NCORES          # images per core
WP = W + 2                 # padded row width (58)
HP = H + 3                 # padded rows (59): 1 top, 2 bottom (tail tap reads)
XF = HP * WP               # padded flat length per image-channel (3422)
OF = H * WP                # padded output flat length (3248)
NT = 7                     # output tiles per (img, cout-chunk)
NFREE = OF // NT           # 464 positions per matmul (>=256 keeps f32r fast)

_CACHE = {}


def _build():
    if "nc" in _CACHE:
        return _CACHE["nc"]
    nc = bacc.Bacc("TRN2", target_bir_lowering=False, debug=False)
    x_d = nc.dram_tensor("x", [BPC, CIN, XF], F32R, kind="ExternalInput").ap()
    w_d = nc.dram_tensor("w", [K * K, CIN, COUT], F32R, kind="ExternalInput").ap()
    b_d = nc.dram_tensor("b", [COUT], F32, kind="ExternalInput").ap()
    o_d = nc.dram_tensor("o", [BPC, COUT, OF], F32, kind="ExternalOutput").ap()

    with tile.TileContext(nc) as tc:
        with (
            tc.tile_pool(name="wp", bufs=1) as wp,
            tc.tile_pool(name="xp", bufs=4) as xp,
            tc.tile_pool(name="op", bufs=2) as op,
            tc.tile_pool(name="pp", bufs=4, space="PSUM") as pp,
        ):
            # weights: [cin-in-chunk, tap, cin_chunk, cout]
            w_t = wp.tile([128, K * K, 2, COUT], F32R)
            for t in range(K * K):
                for ci in range(2):
                    nc.sync.dma_start(
                        out=w_t[:, t, ci, :], in_=w_d[t, ci * 128:(ci + 1) * 128, :]
                    )
            bias_t = wp.tile([128, 2], F32)
            for cc in range(2):
                nc.sync.dma_start(
                    out=bias_t[:, cc:cc + 1], in_=b_d[cc * 128:(cc + 1) * 128]
                )

            for img in range(BPC):
                xs = []
                for ci in range(2):
                    x_t = xp.tile([128, XF], F32R, tag="x")
                    # split the 1.75MB load across 2 DMA queues
                    half = XF // 2
                    nc.sync.dma_start(
                        out=x_t[:, :half],
                        in_=x_d[img, ci * 128:(ci + 1) * 128, :half],
                    )
                    nc.sync.dma_start(
                        out=x_t[:, half:],
                        in_=x_d[img, ci * 128:(ci + 1) * 128, half:],
                    )
                    xs.append(x_t)
                for cc in range(2):
                    o_t = op.tile([128, OF], F32, tag="o")
                    for nt in range(NT):
                        ps = pp.tile([128, NFREE], F32, tag="ps")
                        mm = 0
                        for ci in range(2):
                            for t in range(K * K):
                                kh, kw = divmod(t, K)
                                off = nt * NFREE + kh * WP + kw
                                nc.tensor.matmul(
                                    ps,
                                    w_t[:, t, ci, cc * 128:(cc + 1) * 128],
                                    xs[ci][:, off:off + NFREE],
                                    start=(mm == 0),
                                    stop=(mm == 17),
                                )
                                mm += 1
                        nc.scalar.activation(
                            o_t[:, nt * NFREE:(nt + 1) * NFREE],
                            ps,
                            mybir.ActivationFunctionType.Identity,
                            bias=bias_t[:, cc:cc + 1],
                        )
                    # split the 1.66MB store across 4 DMA queues
                    q = OF // 4
                    for s in range(4):
                        nc.sync.dma_start(
                            out=o_d[img, cc * 128:(cc + 1) * 128, s * q:(s + 1) * q],
                            in_=o_t[:, s * q:(s + 1) * q],
                        )
    nc.compile()
    _CACHE["nc"] = nc
    return nc


def make_in_maps(inp, kernel, bias):
    xpad = np.zeros((B, CIN, HP, WP), np.float32)
    xpad[:, :, 1:1 + H, 1:1 + W] = inp
    xflat = xpad.reshape(B, CIN, XF)
    # [cout, cin, kh, kw] -> [tap(kh*3+kw), cin, cout]
    w_dev = np.ascontiguousarray(
        np.asarray(kernel, np.float32).transpose(2, 3, 1, 0).reshape(K * K, CIN, COUT)
    )
    b_dev = np.ascontiguousarray(np.asarray(bias, np.float32))
    return [
        {"x": np.ascontiguousarray(xflat[c * BPC:(c + 1) * BPC]),
         "w": w_dev, "b": b_dev}
        for c in range(NCORES)
    ]


def assemble(results):
    o = np.concatenate([results[c]["o"] for c in range(NCORES)], axis=0)
    return np.ascontiguousarray(
        o.reshape(B, COUT, H, WP)[:, :, :, :W].astype(np.float32)
    )


def kernel(inp, kernel, bias):
    nc = _build()
    in_maps = make_in_maps(inp, kernel, bias)
    r = run_bass_kernel_spmd(nc, in_maps, core_ids=list(range(NCORES)))
    return assemble(r.results)


# revision 3
# speedup vs baseline: 1.0097x; 1.0097x over previous
"""Trainium2 Bass kernel for nn_Conv2d (B=32, 256->256, 56x56, 3x3, pad=1) + bias.

Strategy
--------
Data-parallel over batch: 4 images per NeuronCore x 8 cores; weights/bias
replicated; no collectives.

Per core, the conv is computed as shifted matmuls: the input is zero-padded on
the HOST to 58-wide rows (59 rows x 58 cols per image-channel, flattened to
3422), so output position (h, w) <-> flat index h*58+w, and the 3x3 tap
(kh, kw) contribution is a matmul against the padded input shifted by the
constant offset kh*58+kw.  Each output tile [128 couts x 464 positions]
accumulates 2 (cin chunks) x 9 (taps) = 18 matmuls in one PSUM bank
(3248 = 7*464 padded output positions per image; columns w in {56,57} are
junk and stripped on the host).  Matmuls run as float32r (1 cycle/row on the
PE at N>=256; ~1.4e-4 relative error, measured on HW).  Bias is fused into
the PSUM->SBUF eviction via ScalarE activation(Identity, bias=...).
"""

import numpy as np

import concourse.bacc as bacc
import concourse.tile as tile
import concourse.mybir as mybir
from concourse.bass_utils import run_bass_kernel_spmd

F32 = mybir.dt.float32
F32R = mybir.dt.float32r

B, CIN, COUT, H, W, K = 32, 256, 256, 56, 56, 3
NCORES = 8
BPC = B // NCORES          # images per core
WP = W + 2                 # padded row width (58)
HP = H + 3                 # padded rows (59): 1 top, 2 bottom (tail tap reads)
XF = HP * WP               # padded flat length per image-channel (3422)
OF = H * WP                # padded output flat length (3248)
NT = 7                     # output tiles per (img, cout-chunk)
NFREE = OF // NT           # 464 positions per matmul (>=256 keeps f32r fast)

_CACHE = {}


def _build():
    if "nc" in _CACHE:
        return _CACHE["nc"]
    nc = bacc.Bacc("TRN2", target_bir_lowering=False, debug=False)
    x_d = nc.dram_tensor("x", [BPC, CIN, XF], F32R, kind="ExternalInput").ap()
    w_d = nc.dram_tensor("w", [K * K, CIN, COUT], F32R, kind="ExternalInput").ap()
    b_d = nc.dram_tensor("b", [COUT], F32, kind="ExternalInput").ap()
    o_d = nc.dram_tensor("o", [BPC, COUT, OF], F32, kind="ExternalOutput").ap()

    with tile.TileContext(nc) as tc:
        with (
            tc.tile_pool(name="wp", bufs=1) as wp,
            tc.tile_pool(name="xp", bufs=6) as xp,
            tc.tile_pool(name="op", bufs=2) as op,
            tc.tile_pool(name="pp", bufs=4, space="PSUM") as pp,
        ):
            bias_t = wp.tile([128, 2], F32)
            for cc in range(2):
                nc.sync.dma_start(
                    out=bias_t[:, cc:cc + 1], in_=b_d[cc * 128:(cc + 1) * 128]
                )
            # weights: [cin-in-chunk, cin_chunk, tap, cout]; DMA in matmul
            # consumption order (ci-major, tap-minor) so compute starts early
            w_t = wp.tile([128, 2, K * K, COUT], F32R)
            for ci in range(2):
                for t in range(K * K):
                    nc.sync.dma_start(
                        out=w_t[:, ci, t, :], in_=w_d[t, ci * 128:(ci + 1) * 128, :]
                    )

            # x loads are sliced so nt-group g only depends on slices <= g+1
            # (sub-tile dep tracking starts matmuls as slices land)
            xsl = [0, 582, 1046, 1510, 1974, 2438, 2902, XF]

            def load_img(img):
                xs = []
                for ci in range(2):
                    x_t = xp.tile([128, XF], F32R, tag="x")
                    xs.append(x_t)
                for s in range(len(xsl) - 1):
                    for ci in range(2):
                        nc.sync.dma_start(
                            out=xs[ci][:, xsl[s]:xsl[s + 1]],
                            in_=x_d[img, ci * 128:(ci + 1) * 128, xsl[s]:xsl[s + 1]],
                        )
                return xs

            for img in range(BPC):
                xs = load_img(img)
                for cc in range(2):
                    o_t = op.tile([128, OF], F32, tag="o")
                    for nt in range(NT):
                        ps = pp.tile([128, NFREE], F32, tag="ps")
                        mm = 0
                        for ci in range(2):
                            for t in range(K * K):
                                kh, kw = divmod(t, K)
                                off = nt * NFREE + kh * WP + kw
                                nc.tensor.matmul(
                                    ps,
                                    w_t[:, ci, t, cc * 128:(cc + 1) * 128],
                                    xs[ci][:, off:off + NFREE],
                                    start=(mm == 0),
                                    stop=(mm == 17),
                                )
                                mm += 1
                        nc.scalar.activation(
                            o_t[:, nt * NFREE:(nt + 1) * NFREE],
                            ps,
                            mybir.ActivationFunctionType.Identity,
                            bias=bias_t[:, cc:cc + 1],
                        )
                        # store each nt slice as soon as its ACT finishes
                        # (2 DMAs each) to keep the final-drain tail short
                        h0 = nt * NFREE
                        h1 = h0 + NFREE // 2
                        nc.sync.dma_start(
                            out=o_d[img, cc * 128:(cc + 1) * 128, h0:h1],
                            in_=o_t[:, h0:h1],
                        )
                        nc.sync.dma_start(
                            out=o_d[img, cc * 128:(cc + 1) * 128, h1:h0 + NFREE],
                            in_=o_t[:, h1:h0 + NFREE],
                        )
    nc.compile()
    _CACHE["nc"] = nc
    return nc


def make_in_maps(inp, kernel, bias):
    xpad = np.zeros((B, CIN, HP, WP), np.float32)
    xpad[:, :, 1:1 + H, 1:1 + W] = inp
    xflat = xpad.reshape(B, CIN, XF)
    # [cout, cin, kh, kw] -> [tap(kh*3+kw), cin, cout]
    w_dev = np.ascontiguousarray(
        np.asarray(kernel, np.float32).transpose(2, 3, 1, 0).reshape(K * K, CIN, COUT)
    )
    b_dev = np.ascontiguousarray(np.asarray(bias, np.float32))
    return [
        {"x": np.ascontiguousarray(xflat[c * BPC:(c + 1) * BPC]),
         "w": w_dev, "b": b_dev}
        for c in range(NCORES)
    ]


def assemble(results):
    o = np.concatenate([results[c]["o"] for c in range(NCORES)], axis=0)
    return np.ascontiguousarray(
        o.reshape(B, COUT, H, WP)[:, :, :, :W].astype(np.float32)
    )


def kernel(inp, kernel, bias):
    nc = _build()
    in_maps = make_in_maps(inp, kernel, bias)
    r = run_bass_kernel_spmd(nc, in_maps, core_ids=list(range(NCORES)))
    return assemble(r.results)
